# revision 16
# baseline (speedup 1.0000x reference)
"""Multi-head causal attention (B=2, T=2048, E=1024, H=16, D=64) on 8 TRN2
NeuronCores, tensor-parallel over heads (2 heads/core).

Dataflow per core (all matmuls fp32r = full-rate reduced-precision fp32):
  host:  xT = x^T  [E, B*T]  (shared);  wqkv_c [E, 384];  wproj_c [128, E]
  1. qT[d,t], kT[d,t], vT[d,t] = wqkv_c^T @ xT     (PSUM accum over E tiles)
     v[s,d] tiles via PE transpose of vT (+ ones column for the softmax sum)
  2. per (b, head, 512-t-block):
       weiT[s,t] = kT^T q  (direct transposed scores, K=64)
       additive causal mask on diagonal 128-chunks, Exp on ACT (scale=1/Ec)
       avT_aug[65,t] = [v|1]^T @ expweiT   (row 64 = softmax denominator)
       recip + PE partition-broadcast; avT_sb = avT * (1/l)  (normalized)
  3. y_partial[t,e] = avT_sb^T @ wproj_c ; DMA out.
  host:  y = sum_c y_partial_c + bproj.
"""
import sys
import types

import numpy as np

B, T, E, H, D = 2, 2048, 1024, 16, 64
N_CORES = 8
HPC = H // N_CORES          # heads per core = 2
BT = B * T                  # 4096
DPC = HPC * D               # 128 head-dims per core
SCALE = 1.0 / float(np.sqrt(E))  # NOTE: reference scales by E**-0.5
NEG = -1e9


def _install_ntff_hook():
    if 'antenv.axon_hooks' in sys.modules:
        return
    try:
        sys.path.insert(0, '/root/.axon_site')
        from trn_agent_boot.trn_boot import _ntff_profile_via_ctypes
        hook = _ntff_profile_via_ctypes('/opt/axon/libaxon_pjrt.so')
        mod = types.ModuleType('antenv.axon_hooks')
        mod.get_axon_ntff_profile_hook = lambda: hook
        mod.set_axon_ntff_profile_hook = lambda h: None
        sys.modules['antenv.axon_hooks'] = mod
    except Exception:
        pass


def _split_multi_waits(nc, mybir):
    """This walrus build rejects >1 sync-wait per instruction. Hoist extra
    waits onto EventSemaphore instructions on the same engine just before."""
    for f in nc.m.functions:
        for bb in f.blocks:
            new_insts = []
            changed = False
            for inst in bb.instructions:
                si = inst.sync_info
                if si is not None and len(si.on_wait) > 1:
                    extra = list(si.on_wait[:-1])
                    keep = si.on_wait[-1]
                    for w in extra:
                        ev = mybir.InstEventSemaphore(
                            name=f"I-{nc.next_id()}", ins=[], outs=[])
                        ev.engine = inst.engine
                        ev.sync_info = mybir.SyncInfo(on_wait=[w], on_update=[])
                        new_insts.append(ev)
                    del si.on_wait[:]
                    si.on_wait.append(keep)
                    changed = True
                new_insts.append(inst)
            if changed:
                bb.instructions = new_insts


def _build_nc():
    import concourse.bass as bass
    import concourse.mybir as mybir
    import concourse.tile as tile
    from concourse.masks import make_identity

    def act_recip(nc, out_ap, in_ap):
        eng = nc.scalar
        imm = lambda v: mybir.ImmediateValue(dtype=mybir.dt.float32, value=v)
        return eng.add_instruction(mybir.InstActivation(
            name=nc.get_next_instruction_name(),
            func=mybir.ActivationFunctionType.Reciprocal,
            ins=[eng.lower_ap(in_ap), imm(0.0), imm(1.0), imm(0.0)],
            outs=[eng.lower_ap(out_ap)]))

    f32 = mybir.dt.float32
    f32r = mybir.dt.float32r
    bf16 = mybir.dt.bfloat16
    EXP = mybir.ActivationFunctionType.Exp

    nc = bass.Bass('TRN2', num_devices=N_CORES)
    xt = nc.dram_tensor('xt', [E, BT], bf16, kind='ExternalInput')
    wqkv = nc.dram_tensor('wqkv', [E, 3 * DPC], bf16, kind='ExternalInput')
    wproj = nc.dram_tensor('wproj', [DPC, E], f32r, kind='ExternalInput')
    y = nc.dram_tensor('y', [BT, E], f32, kind='ExternalOutput')

    NTB1 = BT // 512            # 8 t-blocks in phase 1
    NE = E // 128               # 8 e-tiles
    NTB = T // 512              # 4 t-blocks per batch in phase 2
    NST = T // 128              # 16 s-tiles per batch

    with tile.TileContext(nc) as tc:
        with tc.tile_pool(name='consts', bufs=1) as consts, \
             tc.tile_pool(name='big', bufs=1) as big, \
             tc.tile_pool(name='work', bufs=1) as work, \
             tc.tile_pool(name='ps', bufs=1, space='PSUM') as ps:

            # ---- constants ----
            ident = consts.tile([128, 128], f32)
            make_identity(nc, ident)
            # additive mask for the diagonal chunk of weiT [s,t]:
            # keep (0) where t >= s, NEG where t < s
            tmask_f = consts.tile([128, 128], f32)
            nc.gpsimd.memset(tmask_f[:], 1.0)
            nc.gpsimd.affine_select(
                out=tmask_f[:], in_=tmask_f[:],
                compare_op=mybir.AluOpType.is_ge,
                fill=0.0, base=0, pattern=[[1, 128]], channel_multiplier=-1)
            tmask = consts.tile([128, 128], bf16)
            nc.vector.tensor_copy(tmask[:], tmask_f[:])
            ones_f32 = consts.tile([128, 2, 1], f32)
            nc.gpsimd.memset(ones_f32[:], 1.0)
            # sel [33,128]: row0 -> partitions 0-63 (head0), row32 -> 64-127
            sel_f32 = consts.tile([33, 128], f32)
            nc.gpsimd.memset(sel_f32[:], 0.0)
            nc.gpsimd.memset(sel_f32[0:1, 0:64], 1.0)
            nc.gpsimd.memset(sel_f32[32:33, 64:128], 1.0)
            sel_bc = consts.tile([33, 128], f32r)   # lhsT of bcast matmul
            nc.vector.tensor_copy(sel_bc[:], sel_f32[:])

            # ---- weights ----
            wqkv_sb = [consts.tile([128, 3 * DPC], bf16, name=f'wqkv{k}')
                       for k in range(NE)]
            for k in range(NE):
                nc.sync.dma_start(out=wqkv_sb[k][:], in_=wqkv[k * 128:(k + 1) * 128, :])
            wproj_sb = consts.tile([DPC, E], f32r)
            nc.sync.dma_start(out=wproj_sb[:], in_=wproj[:])

            # ---- persistent activations ----
            qT_sb = [big.tile([128, 512], bf16, name=f'q{j}')
                     for j in range(NTB1)]
            kT_sb = [big.tile([128, 512], bf16, name=f'k{j}')
                     for j in range(NTB1)]
            # v tiles [s,d] per 128-s-tile, layout [128, 2, 65]: per head 64
            # dims + ones column (softmax denominator via matmul)
            v_sb = [big.tile([128, 2, 65], bf16, name=f'v{si}')
                    for si in range(2 * NST)]
            for si in range(2 * NST):
                nc.gpsimd.memset(v_sb[si][:, :, 64:65], 1.0)

            # ================= phase 1: QKV projections =================
            for tb in range(NTB1):
                ts = tb * 512
                q_ps = ps.tile([128, 512], f32, tag='acc', bufs=3)
                k_ps = ps.tile([128, 512], f32, tag='acc', bufs=3)
                vt_ps = ps.tile([128, 512], f32, tag='acc', bufs=3)
                for k in range(NE):
                    xt_t = work.tile([128, 512], bf16, tag='xt', bufs=6)
                    nc.sync.dma_start(
                        out=xt_t[:], in_=xt[k * 128:(k + 1) * 128, ts:ts + 512])
                    st, sp = (k == 0), (k == NE - 1)
                    nc.tensor.matmul(q_ps[:], wqkv_sb[k][:, 0:128], xt_t[:],
                                     start=st, stop=sp)
                    nc.tensor.matmul(k_ps[:], wqkv_sb[k][:, 128:256], xt_t[:],
                                     start=st, stop=sp)
                    nc.tensor.matmul(vt_ps[:], wqkv_sb[k][:, 256:384], xt_t[:],
                                     start=st, stop=sp)
                nc.vector.tensor_copy(qT_sb[tb][:], q_ps[:])
                nc.vector.tensor_copy(kT_sb[tb][:], k_ps[:])
                vt_sb = work.tile([128, 512], f32, tag='vt', bufs=2)
                nc.vector.tensor_copy(vt_sb[:], vt_ps[:])
                # transpose vT -> v [s, d] per 128-chunk
                for sc in range(4):
                    si = tb * 4 + sc
                    vtr = ps.tile([128, 128], f32, tag='misc', bufs=2)
                    nc.tensor.transpose(vtr[:], vt_sb[:, sc * 128:(sc + 1) * 128],
                                        ident[:])
                    nc.vector.tensor_copy(
                        v_sb[si][:, :, 0:64],
                        vtr.rearrange('p (h e) -> p h e', h=2))

            # ============ phase 2+3: attention + projection ============
            for b in range(B):
                for tb in range(NTB):
                    t0 = b * T + tb * 512          # global t offset
                    n_si = 4 * (tb + 1)            # s-tiles (causal)
                    avT_sb = work.tile([128, 512], f32r, tag='avT', bufs=3)
                    rc2 = work.tile([33, 512], f32, tag='rc', bufs=2)
                    nc.gpsimd.memset(rc2[:], 1.0)
                    av_pss = [ps.tile([65, 512], f32, tag='acc', bufs=3,
                                      name=f'av{b}_{tb}_{h}')
                              for h in range(HPC)]
                    for si in range(n_si):
                        s0 = b * T + si * 128
                        sblk, srem = divmod(s0 - b * T, 512)
                        sblk += b * NTB
                        woff = 0
                        if si >= 4 * tb:           # diagonal region
                            woff = (si - 4 * tb) * 128
                        for h in range(HPC):
                            hd = h * 64
                            w_ps = ps.tile([128, 512], f32, tag='wei', bufs=3)
                            nc.tensor.matmul(
                                w_ps[:, woff:512],
                                kT_sb[sblk][hd:hd + 64, srem:srem + 128],
                                qT_sb[b * NTB + tb][hd:hd + 64, woff:512],
                                start=True, stop=True)
                            wt = work.tile([128, 512], bf16, tag='weiT', bufs=24)
                            nc.scalar.activation(wt[:, woff:512], w_ps[:, woff:512],
                                                 EXP, scale=SCALE)
                            if si >= 4 * tb:
                                nc.vector.tensor_mul(wt[:, woff:woff + 128],
                                                     wt[:, woff:woff + 128],
                                                     tmask[:])
                            nc.tensor.matmul(
                                av_pss[h][:, woff:512],
                                v_sb[b * NST + si][:, h, :], wt[:, woff:512],
                                start=(si == 0), stop=(si == n_si - 1))
                    for h in range(HPC):
                        # 1/l directly on ACT into partition 0 / 32
                        act_recip(nc, rc2[32 * h:32 * h + 1, :],
                                  av_pss[h][64:65, :])
                    rc2r = work.tile([33, 512], f32r, tag='rcr', bufs=2)
                    nc.vector.tensor_copy(rc2r[:], rc2[:])
                    bc_ps = ps.tile([128, 512], f32, tag='misc', bufs=2)
                    nc.tensor.matmul(bc_ps[:], sel_bc[:], rc2r[:],
                                     start=True, stop=True)
                    bc_sb = work.tile([128, 512], f32, tag='bcs', bufs=2)
                    nc.vector.tensor_copy(bc_sb[:], bc_ps[:])
                    for h in range(HPC):
                        hd = h * 64
                        nc.vector.tensor_mul(avT_sb[hd:hd + 64, :],
                                             av_pss[h][0:64, :],
                                             bc_sb[hd:hd + 64, :])
                    # ---- projection for this 512-t-block ----
                    for tc4 in range(4):
                        for eb in range(2):
                            y_ps = ps.tile([128, 512], f32, tag='misc', bufs=2)
                            nc.tensor.matmul(
                                y_ps[:],
                                avT_sb[:, tc4 * 128:(tc4 + 1) * 128],
                                wproj_sb[:, eb * 512:(eb + 1) * 512],
                                start=True, stop=True)
                            y_sb = work.tile([128, 512], f32, tag='ysb', bufs=4)
                            nc.vector.tensor_copy(y_sb[:], y_ps[:])
                            nc.sync.dma_start(
                                out=y[t0 + tc4 * 128:t0 + (tc4 + 1) * 128,
                                      eb * 512:(eb + 1) * 512],
                                in_=y_sb[:])

    import concourse.mybir as mybir2
    _split_multi_waits(nc, mybir2)
    return nc


_CACHE = {}


def kernel(x, Wq, Wk, Wv, Wproj, bproj):
    _install_ntff_hook()
    from concourse.bass_utils import run_bass_kernel_spmd

    x = np.asarray(x, dtype=np.float32)
    Wq = np.asarray(Wq, dtype=np.float32)
    Wk = np.asarray(Wk, dtype=np.float32)
    Wv = np.asarray(Wv, dtype=np.float32)
    Wproj = np.asarray(Wproj, dtype=np.float32)
    bproj = np.asarray(bproj, dtype=np.float32)

    if 'nc' not in _CACHE:
        _CACHE['nc'] = _build_nc()
    nc = _CACHE['nc']

    import ml_dtypes
    xT = np.ascontiguousarray(x.reshape(BT, E).T).astype(ml_dtypes.bfloat16)
    in_maps = []
    for c in range(N_CORES):
        h0 = HPC * c
        wqkv_c = np.concatenate(
            [Wq[h0], Wq[h0 + 1], Wk[h0], Wk[h0 + 1], Wv[h0], Wv[h0 + 1]],
            axis=1)                                         # [E, 384]
        wproj_c = np.ascontiguousarray(Wproj[DPC * c: DPC * (c + 1)])
        in_maps.append({'xt': xT,
                        'wqkv': np.ascontiguousarray(
                            wqkv_c.astype(ml_dtypes.bfloat16)),
                        'wproj': wproj_c})

    res = run_bass_kernel_spmd(nc, in_maps, list(range(N_CORES)))
    ysum = np.zeros((BT, E), dtype=np.float64)
    for c in range(N_CORES):
        ysum += res.results[c]['y'].astype(np.float64)
    out = (ysum + bproj.astype(np.float64)).astype(np.float32)
    return out.reshape(B, T, E)


# revision 17
# speedup vs baseline: 1.0993x; 1.0993x over previous
"""Multi-head causal attention (B=2, T=2048, E=1024, H=16, D=64) on 8 TRN2
NeuronCores, tensor-parallel over heads (2 heads/core).

Dataflow per core (all matmuls fp32r = full-rate reduced-precision fp32):
  host:  xT = x^T  [E, B*T]  (shared);  wqkv_c [E, 384];  wproj_c [128, E]
  1. qT[d,t], kT[d,t], vT[d,t] = wqkv_c^T @ xT     (PSUM accum over E tiles)
     v[s,d] tiles via PE transpose of vT (+ ones column for the softmax sum)
  2. per (b, head, 512-t-block):
       weiT[s,t] = kT^T q  (direct transposed scores, K=64)
       additive causal mask on diagonal 128-chunks, Exp on ACT (scale=1/Ec)
       avT_aug[65,t] = [v|1]^T @ expweiT   (row 64 = softmax denominator)
       recip + PE partition-broadcast; avT_sb = avT * (1/l)  (normalized)
  3. y_partial[t,e] = avT_sb^T @ wproj_c ; DMA out.
  host:  y = sum_c y_partial_c + bproj.
"""
import sys
import types

import numpy as np

B, T, E, H, D = 2, 2048, 1024, 16, 64
N_CORES = 8
HPC = H // N_CORES          # heads per core = 2
BT = B * T                  # 4096
DPC = HPC * D               # 128 head-dims per core
SCALE = 1.0 / float(np.sqrt(E))  # NOTE: reference scales by E**-0.5
NEG = -1e9


def _install_ntff_hook():
    if 'antenv.axon_hooks' in sys.modules:
        return
    try:
        sys.path.insert(0, '/root/.axon_site')
        from trn_agent_boot.trn_boot import _ntff_profile_via_ctypes
        hook = _ntff_profile_via_ctypes('/opt/axon/libaxon_pjrt.so')
        mod = types.ModuleType('antenv.axon_hooks')
        mod.get_axon_ntff_profile_hook = lambda: hook
        mod.set_axon_ntff_profile_hook = lambda h: None
        sys.modules['antenv.axon_hooks'] = mod
    except Exception:
        pass


def _split_multi_waits(nc, mybir):
    """This walrus build rejects >1 sync-wait per instruction. Hoist extra
    waits onto EventSemaphore instructions on the same engine just before."""
    for f in nc.m.functions:
        for bb in f.blocks:
            new_insts = []
            changed = False
            for inst in bb.instructions:
                si = inst.sync_info
                if si is not None and len(si.on_wait) > 1:
                    extra = list(si.on_wait[:-1])
                    keep = si.on_wait[-1]
                    for w in extra:
                        ev = mybir.InstEventSemaphore(
                            name=f"I-{nc.next_id()}", ins=[], outs=[])
                        ev.engine = inst.engine
                        ev.sync_info = mybir.SyncInfo(on_wait=[w], on_update=[])
                        new_insts.append(ev)
                    del si.on_wait[:]
                    si.on_wait.append(keep)
                    changed = True
                new_insts.append(inst)
            if changed:
                bb.instructions = new_insts


def _build_nc():
    import concourse.bass as bass
    import concourse.mybir as mybir
    import concourse.tile as tile
    from concourse.masks import make_identity

    def act_recip(nc, out_ap, in_ap):
        eng = nc.scalar
        imm = lambda v: mybir.ImmediateValue(dtype=mybir.dt.float32, value=v)
        return eng.add_instruction(mybir.InstActivation(
            name=nc.get_next_instruction_name(),
            func=mybir.ActivationFunctionType.Reciprocal,
            ins=[eng.lower_ap(in_ap), imm(0.0), imm(1.0), imm(0.0)],
            outs=[eng.lower_ap(out_ap)]))

    f32 = mybir.dt.float32
    f32r = mybir.dt.float32r
    bf16 = mybir.dt.bfloat16
    EXP = mybir.ActivationFunctionType.Exp

    nc = bass.Bass('TRN2', num_devices=N_CORES)
    xt = nc.dram_tensor('xt', [E, BT], bf16, kind='ExternalInput')
    wqkv = nc.dram_tensor('wqkv', [E, 3 * DPC], bf16, kind='ExternalInput')
    wproj = nc.dram_tensor('wproj', [DPC, E], f32r, kind='ExternalInput')
    y = nc.dram_tensor('y', [BT, E], f32, kind='ExternalOutput')

    NTB1 = BT // 512            # 8 t-blocks in phase 1
    NE = E // 128               # 8 e-tiles
    NTB = T // 512              # 4 t-blocks per batch in phase 2
    NST = T // 128              # 16 s-tiles per batch

    with tile.TileContext(nc) as tc:
        with tc.tile_pool(name='consts', bufs=1) as consts, \
             tc.tile_pool(name='big', bufs=1) as big, \
             tc.tile_pool(name='work', bufs=1) as work, \
             tc.tile_pool(name='ps', bufs=1, space='PSUM') as ps:

            # ---- constants ----
            ident = consts.tile([128, 128], f32)
            make_identity(nc, ident)
            # additive mask for the diagonal chunk of weiT [s,t]:
            # keep (0) where t >= s, NEG where t < s
            tmask_f = consts.tile([128, 128], f32)
            nc.gpsimd.memset(tmask_f[:], 1.0)
            nc.gpsimd.affine_select(
                out=tmask_f[:], in_=tmask_f[:],
                compare_op=mybir.AluOpType.is_ge,
                fill=0.0, base=0, pattern=[[1, 128]], channel_multiplier=-1)
            tmask = consts.tile([128, 128], bf16)
            nc.vector.tensor_copy(tmask[:], tmask_f[:])
            ones_f32 = consts.tile([128, 2, 1], f32)
            nc.gpsimd.memset(ones_f32[:], 1.0)
            # sel [33,128]: row0 -> partitions 0-63 (head0), row32 -> 64-127
            sel_f32 = consts.tile([33, 128], f32)
            nc.gpsimd.memset(sel_f32[:], 0.0)
            nc.gpsimd.memset(sel_f32[0:1, 0:64], 1.0)
            nc.gpsimd.memset(sel_f32[32:33, 64:128], 1.0)
            sel_bc = consts.tile([33, 128], f32r)   # lhsT of bcast matmul
            nc.vector.tensor_copy(sel_bc[:], sel_f32[:])

            # ---- weights ----
            wqkv_sb = [consts.tile([128, 3 * DPC], bf16, name=f'wqkv{k}')
                       for k in range(NE)]
            for k in range(NE):
                nc.sync.dma_start(out=wqkv_sb[k][:], in_=wqkv[k * 128:(k + 1) * 128, :])
            wproj_sb = consts.tile([DPC, E], f32r)
            nc.sync.dma_start(out=wproj_sb[:], in_=wproj[:])

            # ---- persistent activations ----
            qT_sb = [big.tile([128, 512], bf16, name=f'q{j}')
                     for j in range(NTB1)]
            kT_sb = [big.tile([128, 512], bf16, name=f'k{j}')
                     for j in range(NTB1)]
            # v tiles [s,d] per 128-s-tile, layout [128, 2, 65]: per head 64
            # dims + ones column (softmax denominator via matmul)
            v_sb = [big.tile([128, 2, 65], bf16, name=f'v{si}')
                    for si in range(2 * NST)]
            for si in range(2 * NST):
                nc.gpsimd.memset(v_sb[si][:, :, 64:65], 1.0)

            # ================= phase 1: QKV projections =================
            for tb in range(NTB1):
                ts = tb * 512
                q_ps = ps.tile([128, 512], f32, tag='acc', bufs=3)
                k_ps = ps.tile([128, 512], f32, tag='acc', bufs=3)
                vt_ps = ps.tile([128, 512], f32, tag='acc', bufs=3)
                for k in range(NE):
                    xt_t = work.tile([128, 512], bf16, tag='xt', bufs=6)
                    nc.sync.dma_start(
                        out=xt_t[:], in_=xt[k * 128:(k + 1) * 128, ts:ts + 512])
                    st, sp = (k == 0), (k == NE - 1)
                    nc.tensor.matmul(q_ps[:], wqkv_sb[k][:, 0:128], xt_t[:],
                                     start=st, stop=sp)
                    nc.tensor.matmul(k_ps[:], wqkv_sb[k][:, 128:256], xt_t[:],
                                     start=st, stop=sp)
                    nc.tensor.matmul(vt_ps[:], wqkv_sb[k][:, 256:384], xt_t[:],
                                     start=st, stop=sp)
                nc.vector.tensor_copy(qT_sb[tb][:], q_ps[:])
                nc.vector.tensor_copy(kT_sb[tb][:], k_ps[:])
                vt_sb = work.tile([128, 512], f32, tag='vt', bufs=2)
                nc.vector.tensor_copy(vt_sb[:], vt_ps[:])
                # transpose vT -> v [s, d] per 128-chunk
                for sc in range(4):
                    si = tb * 4 + sc
                    vtr = ps.tile([128, 128], f32, tag='misc', bufs=2)
                    nc.tensor.transpose(vtr[:], vt_sb[:, sc * 128:(sc + 1) * 128],
                                        ident[:])
                    nc.vector.tensor_copy(
                        v_sb[si][:, :, 0:64],
                        vtr.rearrange('p (h e) -> p h e', h=2))

            # ============ phase 2+3: attention + projection ============
            for b in range(B):
                for tb in range(NTB):
                    t0 = b * T + tb * 512          # global t offset
                    n_si = 4 * (tb + 1)            # s-tiles (causal)
                    avT_sb = work.tile([128, 512], f32r, tag='avT', bufs=3)
                    lrow = work.tile([33, 512], f32, tag='lrow', bufs=2)
                    nc.gpsimd.memset(lrow[:], 1.0)
                    av_pss = [ps.tile([65, 512], f32, tag='acc', bufs=3,
                                      name=f'av{b}_{tb}_{h}')
                              for h in range(HPC)]
                    for si in range(n_si):
                        s0 = b * T + si * 128
                        sblk, srem = divmod(s0 - b * T, 512)
                        sblk += b * NTB
                        woff = 0
                        if si >= 4 * tb:           # diagonal region
                            woff = (si - 4 * tb) * 128
                        for h in range(HPC):
                            hd = h * 64
                            w_ps = ps.tile([128, 512], f32, tag='wei', bufs=3)
                            nc.tensor.matmul(
                                w_ps[:],
                                kT_sb[sblk][hd:hd + 64, srem:srem + 128],
                                qT_sb[b * NTB + tb][hd:hd + 64, :],
                                start=True, stop=True)
                            wt = work.tile([128, 512], bf16, tag='weiT', bufs=24)
                            if woff > 0:
                                nc.gpsimd.memset(wt[:, 0:woff], 0.0)
                            nc.scalar.activation(wt[:, woff:512], w_ps[:, woff:512],
                                                 EXP, scale=SCALE)
                            if si >= 4 * tb:
                                nc.vector.tensor_mul(wt[:, woff:woff + 128],
                                                     wt[:, woff:woff + 128],
                                                     tmask[:])
                            nc.tensor.matmul(
                                av_pss[h][:], v_sb[b * NST + si][:, h, :], wt[:],
                                start=(si == 0), stop=(si == n_si - 1))
                    for h in range(HPC):
                        # stash softmax denominator row (partition 0 / 32)
                        nc.vector.tensor_copy(lrow[32 * h:32 * h + 1, :],
                                              av_pss[h][64:65, :])
                    rc2 = work.tile([33, 512], f32, tag='rc', bufs=2)
                    nc.vector.reciprocal(rc2[:], lrow[:])
                    rc2r = work.tile([33, 512], f32r, tag='rcr', bufs=2)
                    nc.vector.tensor_copy(rc2r[:], rc2[:])
                    bc_ps = ps.tile([128, 512], f32, tag='misc', bufs=2)
                    nc.tensor.matmul(bc_ps[:], sel_bc[:], rc2r[:],
                                     start=True, stop=True)
                    bc_sb = work.tile([128, 512], f32, tag='bcs', bufs=2)
                    nc.vector.tensor_copy(bc_sb[:], bc_ps[:])
                    for h in range(HPC):
                        hd = h * 64
                        nc.vector.tensor_mul(avT_sb[hd:hd + 64, :],
                                             av_pss[h][0:64, :],
                                             bc_sb[hd:hd + 64, :])
                    # ---- projection for this 512-t-block ----
                    for tc4 in range(4):
                        for eb in range(2):
                            y_ps = ps.tile([128, 512], f32, tag='misc', bufs=2)
                            nc.tensor.matmul(
                                y_ps[:],
                                avT_sb[:, tc4 * 128:(tc4 + 1) * 128],
                                wproj_sb[:, eb * 512:(eb + 1) * 512],
                                start=True, stop=True)
                            y_sb = work.tile([128, 512], f32, tag='ysb', bufs=4)
                            nc.vector.tensor_copy(y_sb[:], y_ps[:])
                            nc.sync.dma_start(
                                out=y[t0 + tc4 * 128:t0 + (tc4 + 1) * 128,
                                      eb * 512:(eb + 1) * 512],
                                in_=y_sb[:])

    import concourse.mybir as mybir2
    _split_multi_waits(nc, mybir2)
    return nc


_CACHE = {}


def kernel(x, Wq, Wk, Wv, Wproj, bproj):
    _install_ntff_hook()
    from concourse.bass_utils import run_bass_kernel_spmd

    x = np.asarray(x, dtype=np.float32)
    Wq = np.asarray(Wq, dtype=np.float32)
    Wk = np.asarray(Wk, dtype=np.float32)
    Wv = np.asarray(Wv, dtype=np.float32)
    Wproj = np.asarray(Wproj, dtype=np.float32)
    bproj = np.asarray(bproj, dtype=np.float32)

    if 'nc' not in _CACHE:
        _CACHE['nc'] = _build_nc()
    nc = _CACHE['nc']

    import ml_dtypes
    xT = np.ascontiguousarray(x.reshape(BT, E).T).astype(ml_dtypes.bfloat16)
    in_maps = []
    for c in range(N_CORES):
        h0 = HPC * c
        wqkv_c = np.concatenate(
            [Wq[h0], Wq[h0 + 1], Wk[h0], Wk[h0 + 1], Wv[h0], Wv[h0 + 1]],
            axis=1)                                         # [E, 384]
        wproj_c = np.ascontiguousarray(Wproj[DPC * c: DPC * (c + 1)])
        in_maps.append({'xt': xT,
                        'wqkv': np.ascontiguousarray(
                            wqkv_c.astype(ml_dtypes.bfloat16)),
                        'wproj': wproj_c})

    res = run_bass_kernel_spmd(nc, in_maps, list(range(N_CORES)))
    ysum = np.zeros((BT, E), dtype=np.float64)
    for c in range(N_CORES):
        ysum += res.results[c]['y'].astype(np.float64)
    out = (ysum + bproj.astype(np.float64)).astype(np.float32)
    return out.reshape(B, T, E)
